# revision 7
# baseline (speedup 1.0000x reference)
"""DCellLinear batched-GEMM kernel for 8 TRN2 NeuronCores.

Problem: y[s] = x[s] @ W[s].T + b[s] for 4096 independent subsystems,
x[s]: [64, 128], W[s]: [128, 128] (torch Linear layout), b[s]: [128].
Output: concat over s -> [262144, 128] float32.

Strategy (pure expert parallelism, no collectives; 512 subsystems/core):

Transposed formulation per subsystem: y[s]^T = W[s] @ x[s]^T.
  matmul(out, lhsT, rhs) computes lhsT.T @ rhs, so
    lhsT = W[s]^T  [d_in=128, d_out=128]   (stationary, full 128-wide)
    rhs  = x[s]^T  [d_in=128, batch=64]    (moving, 64 all-useful columns)
    out  = y[s]^T  [d_out=128, batch=64]   in PSUM.
  - No wasted PE streaming and zero PE bias matmuls: bias b[s][o] is a
    per-partition operand of y^T, added during the PSUM->SBUF extraction
    as one DVE tensor_tensor per PSUM bank (bias broadcast stride-0
    along the free axis), with an ACT per-subsystem variant mixed in to
    spread extraction across engines.
  - Chunk-major DRAM images (host marshalling): each chunk's x^T / W^T /
    y^T block is one fully contiguous [128 x cols] region, so every DMA
    is a handful of max-size linear descriptors instead of 128
    per-partition strips.
  - bf16 in/out on the wire (rel-err budget 2e-2 >> bf16 ulp; measured
    3.9e-3): total HBM traffic 33.6 MB/core, the minimum possible.
  - Queue layout (measured: each HWDGE queue sustains only ~210-230GB/s;
    Pool SWDGE ~170GB/s; loads must never sit behind stores in a FIFO):
      qSP  (sync):   x loads (8.4MB) + 1/4 of W loads (4.2MB)
      qACT (scalar): 3/4 of W loads (12.6MB)
      qPool(gpsimd): y stores (8.4MB)
    -> both load queues carry 12.6MB, the measured optimum (~60us quiet).
"""

import numpy as np
from contextlib import ExitStack

import concourse.bass as bass
import concourse.mybir as mybir
from concourse.tile import TileContext
from concourse.bass_utils import run_bass_kernel_spmd

N_SUB, BATCH, D_IN, D_OUT = 4096, 64, 128, 128
N_CORES = 8
S_CORE = N_SUB // N_CORES          # 512 subsystems per core

BF16 = mybir.dt.bfloat16
F32 = mybir.dt.float32

_ID = mybir.ActivationFunctionType.Identity

CH_DEFAULT = 32


def build_nc(passes=1, ch=CH_DEFAULT, sbuf_bufs=12, psum_bufs=8, grp=8,
             xq="sync", wq="sync:1+scalar:3", sq="gpsimd", extract="vv",
             split_waits=True):
    """Build the NEFF.  passes>1 repeats the workload (same I/O) purely
    for slope timing.  ch: subsystems per chunk; grp: per PSUM bank."""
    assert S_CORE % ch == 0 and ch % grp == 0
    nchunk = S_CORE // ch
    xc = ch * BATCH                 # x^T / y^T cols per chunk
    wc = ch * D_OUT                 # W^T cols per chunk

    nc = bass.Bass()
    x_in = nc.declare_dram_parameter(
        "x", [nchunk, D_IN, xc], BF16, isOutput=False)
    w_in = nc.declare_dram_parameter(
        "W", [nchunk, D_IN, wc], BF16, isOutput=False)
    b_in = nc.declare_dram_parameter(
        "b", [D_OUT, S_CORE], F32, isOutput=False)
    y_out = nc.declare_dram_parameter(
        "out", [nchunk, D_OUT, xc], BF16, isOutput=True)

    eng = {"sync": nc.sync, "scalar": nc.scalar, "vector": nc.vector,
           "gpsimd": nc.gpsimd, "tensor": nc.tensor}

    def qsplit(spec):
        """'scalar:3+gpsimd:1' -> [(engine, weight), ...]"""
        out = []
        for part in spec.split("+"):
            name, _, w = part.partition(":")
            out.append((eng[name], float(w) if w else 1.0))
        return out

    def qslices(specs, total, align):
        """[(eng, w)] -> [(eng, lo, hi)] column ranges, align-rounded."""
        tw = sum(w for _, w in specs)
        cuts, acc = [0], 0.0
        for _, w in specs[:-1]:
            acc += w
            cuts.append(int(round(acc / tw * total / align)) * align)
        cuts.append(total)
        return [(e, lo, hi) for (e, _), lo, hi
                in zip(specs, cuts[:-1], cuts[1:]) if hi > lo]

    xspecs, wspecs, sspecs = qsplit(xq), qsplit(wq), qsplit(sq)

    with TileContext(nc) as tc, ExitStack() as ctx:
        consts = ctx.enter_context(tc.tile_pool(name="consts", bufs=1))
        xt_pool = ctx.enter_context(tc.tile_pool(name="xt_pool", bufs=sbuf_bufs))
        wt_pool = ctx.enter_context(tc.tile_pool(name="wt_pool", bufs=sbuf_bufs))
        yt_pool = ctx.enter_context(tc.tile_pool(name="yt_pool", bufs=sbuf_bufs))
        py_pool = ctx.enter_context(
            tc.tile_pool(name="py_pool", bufs=psum_bufs, space="PSUM"))

        # bias rides the store (Pool) queue, idle until the first store --
        # keeps the one-time load off the critical first x/W prefetches.
        bT = consts.tile([128, S_CORE], F32)
        sspecs[0][0].dma_start(out=bT, in_=b_in[:, :])

        ei = 0
        for c in [c for _ in range(passes) for c in range(nchunk)]:
            xt = xt_pool.tile([128, xc], BF16)
            for ld, lo, hi in qslices(xspecs, xc, BATCH):
                ld.dma_start(out=xt[:, lo:hi], in_=x_in[c, :, lo:hi])
            wt = wt_pool.tile([128, wc], BF16)
            for ld, lo, hi in qslices(wspecs, wc, D_OUT):
                ld.dma_start(out=wt[:, lo:hi], in_=w_in[c, :, lo:hi])

            yt = yt_pool.tile([128, xc], BF16)
            for q in range(ch // grp):
                yp = py_pool.tile([128, grp, BATCH], F32)
                for j in range(grp):
                    s = q * grp + j
                    nc.tensor.matmul(
                        yp[:, j, :],
                        wt[:, D_OUT * s:D_OUT * (s + 1)],
                        xt[:, BATCH * s:BATCH * (s + 1)],
                        start=True, stop=True)
                sg0 = c * ch + q * grp
                e = extract[ei % len(extract)]
                ei += 1
                if e == "a":    # per-subsystem ACT bias-add extraction
                    for j in range(grp):
                        nc.scalar.activation(
                            yt[:, BATCH * (q * grp + j):
                               BATCH * (q * grp + j + 1)],
                            yp[:, j, :], _ID,
                            bias=bT[:, sg0 + j:sg0 + j + 1], scale=1.0)
                else:           # one DVE tensor_tensor per PSUM bank
                    # (Pool/GPSIMD cannot read PSUM; ACT has no tensor_tensor)
                    dst = yt[:, BATCH * q * grp:BATCH * (q + 1) * grp]
                    dst = dst.rearrange("p (g b) -> p g b", g=grp)
                    bias = (bT[:, sg0:sg0 + grp].unsqueeze(2)
                            .broadcast_to([128, grp, BATCH]))
                    nc.vector.tensor_add(dst, yp[:, :, :], bias)

            for st, lo, hi in qslices(sspecs, xc, BATCH):
                st.dma_start(out=y_out[c, :, lo:hi], in_=yt[:, lo:hi])

    if split_waits:
        _split_excess_waits(nc)
    return nc


# Walrus codegen allows only one sync-wait slot on engine-compute
# instructions; Tile's scheduler can emit several.  Hoist extras onto
# same-engine NoOps placed right before the instruction (the NX
# sequencer drains waits in order before dispatch, so semantics hold).
_WAIT_EXEMPT = {
    "InstCall", "InstUnconditionalBranch",
    "InstEventSemaphore", "InstISA", "InstHalt",
}


def _split_excess_waits(nc, max_waits=1):
    import concourse.mybir as mybir_
    k = 0
    for f in nc.m.functions:
        for blk in f.blocks:
            out = []
            changed = False
            for inst in blk.instructions:
                si = getattr(inst, "sync_info", None)
                if (si is not None and si.on_wait and len(si.on_wait) > max_waits
                        and type(inst).__name__ not in _WAIT_EXEMPT):
                    waits = list(si.on_wait)
                    for w in waits[:-max_waits]:
                        nop = mybir_.InstNoOp(name=f"I-nopw{k}")
                        k += 1
                        nop.engine = inst.engine
                        nop.sync_info = mybir_.SyncInfo(on_wait=[w], on_update=[])
                        out.append(nop)
                    inst.sync_info = mybir_.SyncInfo(
                        on_wait=waits[-max_waits:], on_update=list(si.on_update))
                    changed = True
                out.append(inst)
            if changed:
                blk.instructions = out
    return nc


_CACHE = {}


def _get_nc():
    if "nc" not in _CACHE:
        _CACHE["nc"] = build_nc()
    return _CACHE["nc"]


def _in_maps(x, W, b, ch=CH_DEFAULT):
    """Host-side marshalling: shard, cast bf16, pre-transpose, chunk-major."""
    import ml_dtypes
    bf = ml_dtypes.bfloat16
    nchunk = S_CORE // ch
    maps = []
    for i in range(N_CORES):
        sl = slice(i * S_CORE, (i + 1) * S_CORE)
        xT = x[sl].reshape(nchunk, ch * BATCH, D_IN).astype(bf)
        xT = np.ascontiguousarray(xT.transpose(0, 2, 1))
        WT = W[sl].reshape(nchunk, ch * D_OUT, D_IN).astype(bf)
        WT = np.ascontiguousarray(WT.transpose(0, 2, 1))
        bT = np.ascontiguousarray(b[sl].T.astype(np.float32))
        maps.append({"x": xT, "W": WT, "b": bT})
    return maps


def _unshard(parts, ch=CH_DEFAULT):
    """parts: per-core [nchunk, 128, ch*64] y^T blocks -> [N_SUB*64, 128]."""
    ys = []
    for p in parts:
        a = np.asarray(p)                     # [nchunk, 128, ch*64]
        ys.append(a.transpose(0, 2, 1).reshape(S_CORE * BATCH, D_OUT))
    return np.concatenate(ys, axis=0).astype(np.float32)


def _unshard_from_stack(arr, ch=CH_DEFAULT):
    """bench-path: arr [N_CORES*nchunk, 128, ch*64] -> [N_SUB*64, 128]."""
    nchunk = S_CORE // ch
    a = np.asarray(arr).reshape(N_CORES, nchunk, D_OUT, ch * BATCH)
    return np.concatenate(
        [a[i].transpose(0, 2, 1).reshape(S_CORE * BATCH, D_OUT)
         for i in range(N_CORES)], axis=0)


def _run(x, W, b, trace=False, **kw):
    x = np.asarray(x, dtype=np.float32)
    W = np.asarray(W, dtype=np.float32)
    b = np.asarray(b, dtype=np.float32)
    maps = _in_maps(x, W, b)
    for attempt in range(3):   # retry transient NCC/device flakes
        try:
            res = run_bass_kernel_spmd(
                _get_nc(), maps, core_ids=list(range(N_CORES)),
                trace=trace, **kw)
            break
        except Exception:
            if attempt == 2:
                raise
            import time
            time.sleep(3)
    y = _unshard([res.results[i]["out"] for i in range(N_CORES)])
    return y, res


def kernel(x, W, b):
    y, _ = _run(x, W, b, trace=False)
    return y
